# revision 1
# baseline (speedup 1.0000x reference)
"""Bass/Trainium2 kernel for 3-layer GAT over 8 NeuronCores.

Sharding: core 2b+h handles (batch b, dst-half h). Within a core:
  - Dense "table" matmuls produce per-node rows [edst|esrc|h] for both halves
    (T_H0/T_H1 for src-gathers) plus a duplicate own-half table (T_OWN for
    dst-gathers) so the program is identical on every core.
  - Edges (dst-sorted, self-loops added) are packed into PAIRED bins of
    128 edge slots each: bin A holds a segment's half-0 sources, bin B its
    half-1 sources, <=15 segments per pair (slot 15 = dummy). The pair's two
    matmuls accumulate in PSUM. Slot ids are "inflated" (16*pair+slot), so
    aggregation output columns are written contiguously - no scatter anywhere.
  - Per 32-pair gather batch: dma_gather by src from T_H0 (bin-A edges) and
    T_H1 (bin-B), plus one dma_gather by dst from T_OWN. Attention
    s = exp(leaky_relu(esrc+edst)) folds into the aggregation matmul
    out[c,slot] = sum_e rhs[e,c]*onehot(slot_e), rhs = [s*h | s], giving
    weighted sums + denominators per slot. Softmax max-subtraction is skipped
    (logits are O(1); exp stays in fp32 range) and matches the reference to
    float rounding.
  - Normalization happens in the transposed layout: denominator reciprocal is
    expanded across head blocks with a tiny PE matmul, bias+relu fuse into one
    ACT op. Output columns feed the next layer's table matmul directly
    (already transposed); halves exchange via pairwise AllGather.
"""

import numpy as np

import concourse.bass as bass
import concourse.tile as tile
from concourse import bacc, mybir
from concourse.bass_utils import run_bass_kernel_spmd

F32 = mybir.dt.float32
I16 = mybir.dt.int16

NEG_SLOPE = 0.2
EDGE_DEPTH = 0
EPS = 1e-16
P = 128
NSEG_MAX = 15          # segments per pair (slot 15 reserved for dummies)
PAIRS_PER_BATCH = 32   # 4 supertiles of 8 pairs
N_QUEUES = 4

# Problem dims (hardcoded per the task contract)
N_NODES = 50000
B = 4
F_IN = 128
H, C = 4, 16
HC = H * C            # 64
N_CLS = 16


# ----------------------------------------------------------------------------
# Host preprocessing
# ----------------------------------------------------------------------------

def _pack_half(src, dst, n_lo, n_hi, half):
    """Pack one dst-half's edges into paired bins.

    Returns dict with per-pair arrays:
      srcA/srcB [np_, 128] global src node ids (bin A: src in half0)
      dstv  [np_, 2, 128] dst node ids per (A|B, position)
      slotA/slotB [np_, 128] slot in 0..15 (15 = dummy)
      seg_node [np_, 16] dst node id of each slot (-1 unused)
    """
    half_n = n_hi - n_lo
    sel = (dst >= n_lo) & (dst < n_hi)
    s_, d_ = src[sel], dst[sel]
    order = np.argsort(d_, kind="stable")
    s_, d_ = s_[order], d_[order]
    uniq, seg_start = np.unique(d_, return_index=True)
    assert len(uniq) == half_n, "self-loops guarantee every node is a dst"
    seg_len = np.diff(np.append(seg_start, len(d_)))
    a_side = s_ < N_HALF_GLOBAL[0]  # bin A: src in global half 0
    pairs = []  # list of (list of seg ids)
    cur, curA, curB = [], 0, 0
    for i in range(half_n):
        a0, L = seg_start[i], seg_len[i]
        la = int(a_side[a0 : a0 + L].sum())
        lb = int(L - la)
        if len(cur) >= NSEG_MAX or curA + la > P or curB + lb > P:
            pairs.append(cur)
            cur, curA, curB = [], 0, 0
        cur.append(i)
        curA += la
        curB += lb
    if cur:
        pairs.append(cur)
    np_real = len(pairs)
    out = dict(np_real=np_real)
    npad = -(-np_real // PAIRS_PER_BATCH) * PAIRS_PER_BATCH
    srcA = np.zeros((npad, P), np.int64)
    srcB = np.full((npad, P), N_HALF_GLOBAL[0], np.int64)  # valid half-1 id
    dstv = np.full((npad, 2, P), n_lo, np.int64)  # valid own-half id
    slotA = np.full((npad, P), NSEG_MAX, np.float32)
    slotB = np.full((npad, P), NSEG_MAX, np.float32)
    seg_node = np.full((npad, 16), -1, np.int64)
    for k, segs in enumerate(pairs):
        ea = eb = 0
        for s_i, seg in enumerate(segs):
            a0, L = seg_start[seg], seg_len[seg]
            e_src = s_[a0 : a0 + L]
            e_a = e_src[a_side[a0 : a0 + L]]
            e_b = e_src[~a_side[a0 : a0 + L]]
            la, lb = len(e_a), len(e_b)
            srcA[k, ea : ea + la] = e_a
            slotA[k, ea : ea + la] = s_i
            dstv[k, 0, ea : ea + la] = uniq[seg]
            srcB[k, eb : eb + lb] = e_b
            slotB[k, eb : eb + lb] = s_i
            dstv[k, 1, eb : eb + lb] = uniq[seg]
            seg_node[k, s_i] = uniq[seg]
            ea += la
            eb += lb
    out.update(srcA=srcA, srcB=srcB, dstv=dstv, slotA=slotA, slotB=slotB,
               seg_node=seg_node, npad=npad)
    return out


N_HALF_GLOBAL = [None]


def preprocess(edge_index, n_nodes):
    src = np.asarray(edge_index[0], np.int64)
    dst = np.asarray(edge_index[1], np.int64)
    loop = np.arange(n_nodes, dtype=np.int64)
    src = np.concatenate([src, loop])
    dst = np.concatenate([dst, loop])
    half = n_nodes // 2
    N_HALF_GLOBAL[0] = half
    packs = [_pack_half(src, dst, 0, half, 0),
             _pack_half(src, dst, half, n_nodes, 1)]
    npairs = max(p["npad"] for p in packs)
    npairs = -(-npairs // PAIRS_PER_BATCH) * PAIRS_PER_BATCH
    infl = 16 * npairs
    assert infl <= 32768, f"inflated id space {infl} exceeds int16 range"
    node_pad = -(-half // P) * P
    assert node_pad <= 32768
    for h, pk in enumerate(packs):
        k = npairs - pk["npad"]
        if k:
            for name, fill in [("srcA", 0), ("srcB", half), ("dstv", h * half),
                               ("slotA", NSEG_MAX), ("slotB", NSEG_MAX),
                               ("seg_node", -1)]:
                arr = pk[name]
                pad_shape = (k,) + arr.shape[1:]
                pk[name] = np.concatenate(
                    [arr, np.full(pad_shape, fill, arr.dtype)])
        # inflated id of each node (as a dst in its half)
        inv = np.full(half, -1, np.int64)
        sn = pk["seg_node"].reshape(-1)
        valid = sn >= 0
        inv[sn[valid] - h * half] = np.nonzero(valid)[0]
        assert (inv >= 0).all()
        pk["infl_of_node"] = inv  # [half] -> inflated id
    return dict(packs=packs, npairs=npairs, infl=infl, half=half,
                node_pad=node_pad, n_batches=npairs // PAIRS_PER_BATCH)


def _wrap_idx(flat):
    """dma_gather int16 index layout: idx i at [i%16, i//16], replicated to
    128 partitions."""
    n = len(flat)
    assert n % 16 == 0
    w = np.asarray(flat, np.int64).reshape(n // 16, 16).T
    assert w.max() < 32768 and w.min() >= -32768
    return np.tile(w.astype(np.int16), (8, 1))


def build_core_idx_arrays(pp, h):
    """Per-core (half h) gather index/slot arrays for all batches.

    Layer-1 ids: half-local original node ids (src: in src's half;
    dst: in own half). Layer-2/3 ids: inflated ids (in the resp. half).
    Returns dict of arrays keyed by input-tensor name.
    """
    pk = pp["packs"][h]
    half = pp["half"]
    nb = pp["n_batches"]
    E_BLK = PAIRS_PER_BATCH * P  # 4096
    srcA = pk["srcA"].reshape(nb, E_BLK)
    srcB = pk["srcB"].reshape(nb, E_BLK)
    # dst-gather landing blocks: [A-pair0..31, B-pair0..31] per batch
    dstv = (pk["dstv"].reshape(nb, PAIRS_PER_BATCH, 2, P)
            .transpose(0, 2, 1, 3).reshape(nb, 2 * E_BLK))
    inflS = [pp["packs"][0]["infl_of_node"], pp["packs"][1]["infl_of_node"]]

    def loc(ids, src_half):
        return ids - src_half * half

    def infl_map(ids, src_half):
        return inflS[src_half][ids - src_half * half]

    out = {}
    for tag, f in [("1", loc), ("2", infl_map)]:
        out[f"srcA{tag}"] = np.stack([_wrap_idx(f(srcA[i], 0)) for i in range(nb)])
        out[f"srcB{tag}"] = np.stack([_wrap_idx(f(srcB[i], 1)) for i in range(nb)])
        out[f"dst{tag}"] = np.stack([_wrap_idx(f(dstv[i], h)) for i in range(nb)])
    out["slotA"] = pk["slotA"].reshape(nb, PAIRS_PER_BATCH, P).transpose(0, 2, 1).copy()
    out["slotB"] = pk["slotB"].reshape(nb, PAIRS_PER_BATCH, P).transpose(0, 2, 1).copy()
    return out


def augment_weights(W, a_s, a_d):
    """[F, HC] weights -> [F, 2H + HC] table weights, cols [edst|esrc|h]."""
    Hh, Cc = a_s.shape
    W64 = np.asarray(W, np.float64)
    As = np.zeros((Hh * Cc, Hh))
    Ad = np.zeros((Hh * Cc, Hh))
    for hh in range(Hh):
        As[hh * Cc : (hh + 1) * Cc, hh] = np.asarray(a_s, np.float64)[hh]
        Ad[hh * Cc : (hh + 1) * Cc, hh] = np.asarray(a_d, np.float64)[hh]
    return np.concatenate([W64 @ Ad, W64 @ As, W64], axis=1).astype(np.float32)


# ----------------------------------------------------------------------------
# Bass program
# ----------------------------------------------------------------------------

def build_program(node_pad, infl, n_batches, n_devices=8, mock_collective=False, stop_after=None):
    """Build the SPMD bass program (identical on all cores)."""
    nc = bacc.Bacc("TRN2", target_bir_lowering=False, debug=False,
                   num_devices=n_devices)
    NB = n_batches
    E_BLK = PAIRS_PER_BATCH * P          # edges per side per batch (4096)
    IDXC = E_BLK // 16                   # idx cols for 4096 idxs (256)
    GW = HC + H                          # 68: [s*h | s] matmul lhs cols
    TC12 = 128                           # L1/2 table row stride (512B)
    TC3 = 64                             # L3 table row stride (256B)
    L3W = 2 * N_CLS + 1                  # L3 psum rows: [s*h(16)|pad(16)|s]

    ins = {}

    def inp(name, shape, dtype=F32):
        ins[name] = nc.dram_tensor(name, list(shape), dtype,
                                   kind="ExternalInput")
        return ins[name]

    xT = inp("xT", [P, 2 * node_pad])           # both halves, transposed
    xT_own = inp("xT_own", [P, node_pad])       # own half copy
    W1a = inp("W1a", [F_IN, 2 * H + HC])
    W2a = inp("W2a", [HC, 2 * H + HC])
    W3a = inp("W3a", [HC, 2 + N_CLS])
    b12T = [inp("b1T", [HC, 1]), inp("b2T", [HC, 1])]
    b3T = inp("b3T", [N_CLS, 1])
    E4p = inp("E4p", [GW, HC])                  # rows 64..67 = head indicator
    E1p = inp("E1p", [L3W, N_CLS])              # row 32 = ones
    iota = inp("iota", [P, 16])
    for t in ("1", "2"):
        inp(f"srcA{t}", [NB, P, IDXC], I16)
        inp(f"srcB{t}", [NB, P, IDXC], I16)
        inp(f"dst{t}", [NB, P, 2 * IDXC], I16)
    inp("slotA", [NB, P, PAIRS_PER_BATCH])
    inp("slotB", [NB, P, PAIRS_PER_BATCH])
    outT = nc.dram_tensor("outT", [N_CLS, infl], F32, kind="ExternalOutput")

    GROUPS = [[2 * b_ + 0, 2 * b_ + 1] for b_ in range(n_devices // 2)]

    with tile.TileContext(nc) as tc:
        with (
            tc.tile_pool(name="dram", bufs=1, space="DRAM") as dp,
            tc.tile_pool(name="const", bufs=1) as cp,
            tc.tile_pool(name="mm", bufs=3) as mp,
            tc.tile_pool(name="edge", bufs=2) as ep,
            tc.tile_pool(name="norm", bufs=2) as np_,
            tc.tile_pool(name="psm", bufs=2, space="PSUM") as ps_m,
            tc.tile_pool(name="pse", bufs=4, space="PSUM") as ps_e,
            tc.tile_pool(name="psx", bufs=2, space="PSUM") as ps_x,
        ):
            # DRAM intermediates
            T1 = [dp.tile([node_pad, TC12], F32, tag=f"T1{h}", name=f"T1{h}") for h in range(2)]
            T1o = dp.tile([node_pad, TC3], F32, tag="T1o")
            T2 = [dp.tile([infl, TC12], F32, tag=f"T2{h}", name=f"T2{h}") for h in range(2)]
            T2o = dp.tile([infl, TC3], F32, tag="T2o")
            T3 = [dp.tile([infl, TC3], F32, tag=f"T3{h}", name=f"T3{h}") for h in range(2)]
            T3o = dp.tile([infl, TC3], F32, tag="T3o")
            xn = [dp.tile([HC, infl], F32, tag=f"xn{l}", name=f"xn{l}") for l in range(2)]
            xnf = [dp.tile([2 * HC, infl], F32, tag=f"xnf{l}", name=f"xnf{l}") for l in range(2)]

            # constants
            w1_t = cp.tile([F_IN, 2 * H + HC], F32)
            w2_t = cp.tile([HC, 2 * H + HC], F32)
            w3_t = cp.tile([HC, 2 + N_CLS], F32)
            b1_t = cp.tile([HC, 1], F32)
            b2_t = cp.tile([HC, 1], F32)
            b3_t = cp.tile([N_CLS, 1], F32)
            e4_t = cp.tile([GW, HC], F32)
            e1_t = cp.tile([L3W, N_CLS], F32)
            io_t = cp.tile([P, 16], F32)
            for t_, d_ in [(w1_t, W1a), (w2_t, W2a), (w3_t, W3a),
                           (b1_t, ins["b1T"]), (b2_t, ins["b2T"]),
                           (b3_t, b3T), (e4_t, E4p), (e1_t, E1p),
                           (io_t, iota)]:
                nc.sync.dma_start(out=t_[:], in_=d_[:, :])

            def phase_m(dst_tables, own_table, src_full, src_own, w_t, tcols,
                        kdim):
                """Dense table matmuls, 4 node-tiles per DMA/PSUM round.
                src_full: AP-maker f(h) -> [kdim, *]."""
                wcols = w_t.shape[1]
                n_t = dst_tables[0].shape[0] // P
                assert n_t % 4 == 0
                MB = 4 * P

                def quad(src_ap, q, table, cols):
                    xc = mp.tile([kdim, MB], F32, tag="xc")
                    nc.sync.dma_start(out=xc[:],
                                      in_=src_ap[:, q * MB : (q + 1) * MB])
                    psm = ps_m.tile([P, 4 * wcols], F32, space="PSUM",
                                    tag="psm")
                    for j in range(4):
                        nc.tensor.matmul(
                            out=psm[:, j * wcols : (j + 1) * wcols],
                            lhsT=xc[:, j * P : (j + 1) * P], rhs=w_t[:],
                            start=True, stop=True)
                    sb = mp.tile([P, 4 * cols], F32, tag=f"msb{cols}")
                    cc = min(wcols, cols)
                    if cc < cols:
                        nc.vector.memset(sb[:], 0.0)
                    nc.vector.tensor_copy(
                        out=sb[:].rearrange("p (j c) -> p j c", c=cols)[
                            :, :, :cc],
                        in_=psm[:].rearrange("p (j c) -> p j c", c=wcols)[
                            :, :, :cc])
                    nc.sync.dma_start(
                        out=table[:][q * MB : (q + 1) * MB, :].rearrange(
                            "(j r) c -> r j c", j=4),
                        in_=sb[:].rearrange("p (j c) -> p j c", c=cols))

                for h in range(2):
                    for q in range(n_t // 4):
                        quad(src_full(h), q, dst_tables[h], tcols)
                for q in range(n_t // 4):
                    quad(src_own, q, own_table, TC3)

            def edge_phase(layer, tabs, tab_own, telem, idx_tag, nheads,
                           ncols, bias_t, exp_t, relu, out_cols, out_dst):
                """One layer's edge aggregation.

                telem: src-gather elem size (row elements gathered)
                nheads: H or 1; ncols: HC or N_CLS
                out_cols/out_dst: per-supertile output row count + DRAM tile
                """
                eoff = nheads          # esrc col offset in gathered row
                hoff = 2 * nheads      # h cols offset
                rw = ncols * nheads // nheads  # = ncols
                if layer < 3:
                    RW = GW            # rhs cols per pair [s*h 64 | s 4]
                    dn0, dn1 = HC, GW
                else:
                    RW = L3W           # [s*h 16 | zeros 16 | s 1]
                    dn0, dn1 = 2 * N_CLS, L3W
                for k in range(NB):
                    sA = ep.tile([P, IDXC], I16, tag="sA")
                    sB = ep.tile([P, IDXC], I16, tag="sB")
                    sD = ep.tile([P, 2 * IDXC], I16, tag="sD")
                    slA = ep.tile([P, PAIRS_PER_BATCH], F32, tag="slA")
                    slB = ep.tile([P, PAIRS_PER_BATCH], F32, tag="slB")
                    nc.sync.dma_start(out=sA[:], in_=ins[f"srcA{idx_tag}"][k])
                    nc.sync.dma_start(out=sB[:], in_=ins[f"srcB{idx_tag}"][k])
                    nc.sync.dma_start(out=sD[:], in_=ins[f"dst{idx_tag}"][k])
                    nc.sync.dma_start(out=slA[:], in_=ins["slotA"][k])
                    nc.sync.dma_start(out=slB[:], in_=ins["slotB"][k])
                    gA = ep.tile([P, PAIRS_PER_BATCH * telem], F32, tag="gA")
                    gB = ep.tile([P, PAIRS_PER_BATCH * telem], F32, tag="gB")
                    gD = ep.tile([P, 2 * PAIRS_PER_BATCH * TC3], F32, tag="gD")
                    GCH = 1024  # ring capacity limit per dma_gather
                    for g0 in range(0, E_BLK, GCH):
                        blk = slice(g0 // P, (g0 + GCH) // P)
                        col = slice(g0 // 16, (g0 + GCH) // 16)
                        nc.gpsimd.dma_gather(
                            out_ap=gA[:].rearrange("p (g c) -> p g c",
                                                   c=telem)[:, blk, :],
                            in_ap=tabs[0][:][:, :telem], idxs_ap=sA[:, col],
                            num_idxs=GCH, num_idxs_reg=GCH, elem_size=telem,
                            elem_step=tabs[0].shape[1], queue_num=0)
                        nc.gpsimd.dma_gather(
                            out_ap=gB[:].rearrange("p (g c) -> p g c",
                                                   c=telem)[:, blk, :],
                            in_ap=tabs[1][:][:, :telem], idxs_ap=sB[:, col],
                            num_idxs=GCH, num_idxs_reg=GCH, elem_size=telem,
                            elem_step=tabs[1].shape[1], queue_num=0)
                    for g0 in range(0, 2 * E_BLK, GCH):
                        blk = slice(g0 // P, (g0 + GCH) // P)
                        col = slice(g0 // 16, (g0 + GCH) // 16)
                        nc.gpsimd.dma_gather(
                            out_ap=gD[:].rearrange("p (g c) -> p g c",
                                                   c=TC3)[:, blk, :],
                            in_ap=tab_own[:][:, :TC3], idxs_ap=sD[:, col],
                            num_idxs=GCH, num_idxs_reg=GCH, elem_size=TC3,
                            elem_step=TC3, queue_num=0)
                    if EDGE_DEPTH == 1:
                        continue
                    rhs = {}
                    ind = {}
                    for side, gS, slS, dblk in (("a", gA, slA, 0),
                                                ("b", gB, slB, 1)):
                        gv = gS[:].rearrange("p (g c) -> p g c", c=telem)
                        dv = gD[:].rearrange("p (g c) -> p g c", c=TC3)[
                            :, dblk * PAIRS_PER_BATCH : (dblk + 1) * PAIRS_PER_BATCH, :]
                        z = ep.tile([P, PAIRS_PER_BATCH * nheads], F32,
                                    tag=f"z{side}")
                        nc.vector.tensor_tensor(
                            out=z[:].rearrange("p (g h) -> p g h", h=nheads),
                            in0=gv[:, :, eoff : eoff + nheads],
                            in1=dv[:, :, 0:nheads], op=mybir.AluOpType.add)
                        zs = ep.tile([P, PAIRS_PER_BATCH * nheads], F32,
                                     tag=f"zs{side}")
                        nc.vector.tensor_scalar_mul(out=zs[:], in0=z[:],
                                                    scalar1=NEG_SLOPE)
                        nc.vector.tensor_tensor(out=z[:], in0=z[:], in1=zs[:],
                                                op=mybir.AluOpType.max)
                        s_t = ep.tile([P, PAIRS_PER_BATCH * nheads], F32,
                                      tag=f"s{side}")
                        nc.scalar.activation(
                            out=s_t[:], in_=z[:],
                            func=mybir.ActivationFunctionType.Exp)
                        r = ep.tile([P, PAIRS_PER_BATCH * RW], F32,
                                    tag=f"r{side}")
                        rv = r[:].rearrange("p (g c) -> p g c", c=RW)
                        sv = s_t[:].rearrange("p (g h) -> p g h", h=nheads)
                        nc.vector.tensor_tensor(
                            out=rv[:, :, : nheads * C].rearrange(
                                "p g (h c) -> p g h c", c=C),
                            in0=gv[:, :, hoff : hoff + nheads * C].rearrange(
                                "p g (h c) -> p g h c", c=C),
                            in1=sv.unsqueeze(3).broadcast_to(
                                [P, PAIRS_PER_BATCH, nheads, C]),
                            op=mybir.AluOpType.mult)
                        if layer == 3:
                            nc.vector.memset(
                                rv[:, :, N_CLS : 2 * N_CLS], 0.0)
                        nc.vector.tensor_copy(out=rv[:, :, dn0:dn1], in_=sv)
                        it = ep.tile([P, PAIRS_PER_BATCH * 16], F32,
                                     tag=f"i{side}")
                        nc.vector.tensor_tensor(
                            out=it[:].rearrange("p (g i) -> p g i", i=16),
                            in0=slS[:].unsqueeze(2).broadcast_to(
                                [P, PAIRS_PER_BATCH, 16]),
                            in1=io_t[:].unsqueeze(1).broadcast_to(
                                [P, PAIRS_PER_BATCH, 16]),
                            op=mybir.AluOpType.is_equal)
                        rhs[side] = r
                        ind[side] = it
                    if EDGE_DEPTH == 2:
                        continue
                    for st in range(PAIRS_PER_BATCH // 8):
                        acc = ps_e.tile([RW, P], F32, space="PSUM", tag="acc")
                        for j in range(8):
                            g = st * 8 + j
                            nc.tensor.matmul(
                                out=acc[:, 16 * j : 16 * (j + 1)],
                                lhsT=rhs["a"][:, RW * g : RW * (g + 1)],
                                rhs=ind["a"][:, 16 * g : 16 * (g + 1)],
                                start=True, stop=False)
                            nc.tensor.matmul(
                                out=acc[:, 16 * j : 16 * (j + 1)],
                                lhsT=rhs["b"][:, RW * g : RW * (g + 1)],
                                rhs=ind["b"][:, 16 * g : 16 * (g + 1)],
                                start=False, stop=True)
                        if EDGE_DEPTH == 3:
                            continue
                        seg = np_.tile([dn1, P], F32, tag="seg")
                        nc.vector.tensor_copy(out=seg[:], in_=acc[:])
                        nc.vector.tensor_scalar_add(
                            out=seg[dn0 : dn0 + nheads, :],
                            in0=seg[dn0 : dn0 + nheads, :], scalar1=EPS)
                        rec = np_.tile([dn1, P], F32, tag="rec")
                        nc.vector.reciprocal(out=rec[dn0 : dn0 + nheads, :],
                                             in_=seg[dn0 : dn0 + nheads, :])
                        rxp = ps_x.tile([out_cols, P], F32, space="PSUM",
                                        tag="rxp")
                        nc.tensor.matmul(out=rxp[:],
                                         lhsT=exp_t[dn0 : dn0 + nheads, :],
                                         rhs=rec[dn0 : dn0 + nheads, :],
                                         start=True, stop=True)
                        rex = np_.tile([out_cols, P], F32, tag="rex")
                        nc.vector.tensor_copy(out=rex[:], in_=rxp[:])
                        o_t = np_.tile([out_cols, P], F32, tag="o")
                        nc.vector.tensor_tensor(out=o_t[:],
                                                in0=seg[:out_cols, :],
                                                in1=rex[:],
                                                op=mybir.AluOpType.mult)
                        nc.scalar.activation(
                            out=o_t[:], in_=o_t[:],
                            func=(mybir.ActivationFunctionType.Relu if relu
                                  else mybir.ActivationFunctionType.Identity),
                            bias=bias_t[:, :1], scale=1.0)
                        col0 = (k * (PAIRS_PER_BATCH // 8) + st) * P
                        nc.sync.dma_start(
                            out=out_dst[:, col0 : col0 + P], in_=o_t[:])

            # ---------------- layer 1 ----------------
            phase_m(T1, T1o, lambda h: xT[:, h * node_pad : (h + 1) * node_pad],
                    xT_own[:, :], w1_t, TC12, F_IN)
            stop = [False]

            def _chk(tag):
                if stop[0] or stop_after == tag:
                    stop[0] = True
                return stop[0]

            if not _chk("m1"):
                edge_phase(1, T1, T1o, TC12, "1", H, HC, b1_t, e4_t, True,
                           HC, xn[0][:])
            if not _chk("e1"):
                if mock_collective:
                    nc.sync.dma_start(out=xnf[0][:][:HC, :], in_=xn[0][:][:, :])
                    nc.sync.dma_start(out=xnf[0][:][HC:, :], in_=xn[0][:][:, :])
                else:
                    nc.gpsimd.collective_compute(
                        "AllGather", mybir.AluOpType.bypass,
                        replica_groups=GROUPS,
                        ins=[xn[0][:][:, :]], outs=[xnf[0][:][:, :]])
            # ---------------- layer 2 ----------------
            xnf0 = xnf[0][:]
            if not _chk("x1"):
                phase_m(T2, T2o,
                        lambda h: xnf0[h * HC : (h + 1) * HC, :],
                        xn[0][:][:, :], w2_t, TC12, HC)
            if not _chk("m2"):
                edge_phase(2, T2, T2o, TC12, "2", H, HC, b2_t, e4_t, True,
                           HC, xn[1][:])
            if not _chk("e2"):
                if mock_collective:
                    nc.sync.dma_start(out=xnf[1][:][:HC, :], in_=xn[1][:][:, :])
                    nc.sync.dma_start(out=xnf[1][:][HC:, :], in_=xn[1][:][:, :])
                else:
                    nc.gpsimd.collective_compute(
                        "AllGather", mybir.AluOpType.bypass,
                        replica_groups=GROUPS,
                        ins=[xn[1][:][:, :]], outs=[xnf[1][:][:, :]])
            # ---------------- layer 3 ----------------
            xnf1 = xnf[1][:]
            if not _chk("x2"):
                phase_m(T3, T3o,
                        lambda h: xnf1[h * HC : (h + 1) * HC, :],
                        xn[1][:][:, :], w3_t, TC3, HC)
            if not _chk("m3"):
                edge_phase(3, T3, T3o, TC3, "2", 1, N_CLS, b3_t, e1_t, False,
                           N_CLS, outT[:, :])

    nc.compile()
    return nc


# ----------------------------------------------------------------------------
# Entry point
# ----------------------------------------------------------------------------

BUILD_KWARGS = {}

def kernel(**inputs):
    x = np.asarray(inputs["x"], np.float32)
    edge_index = np.asarray(inputs["edge_index"])
    Bc, Nn, Fi = x.shape
    pp = preprocess(edge_index, Nn)
    half, node_pad, infl = pp["half"], pp["node_pad"], pp["infl"]

    W1a = augment_weights(inputs["W1"], inputs["a1s"], inputs["a1d"])
    W2a = augment_weights(inputs["W2"], inputs["a2s"], inputs["a2d"])
    W3a = augment_weights(inputs["W3"], inputs["a3s"], inputs["a3d"])
    b1 = np.asarray(inputs["b1"], np.float32).reshape(HC, 1)
    b2 = np.asarray(inputs["b2"], np.float32).reshape(HC, 1)
    b3 = np.asarray(inputs["b3"], np.float32).reshape(N_CLS, 1)
    GW = HC + H
    L3W = 2 * N_CLS + 1
    E4p = np.zeros((GW, HC), np.float32)
    for hh in range(H):
        E4p[HC + hh, hh * C : (hh + 1) * C] = 1.0
    E1p = np.zeros((L3W, N_CLS), np.float32)
    E1p[2 * N_CLS, :] = 1.0
    iota = np.broadcast_to(np.arange(16, dtype=np.float32), (P, 16)).copy()

    nc = build_program(node_pad, infl, pp["n_batches"], n_devices=8,
                       **BUILD_KWARGS)

    # per-core inputs
    idx_half = [build_core_idx_arrays(pp, h) for h in range(2)]
    in_maps = []
    for c in range(8):
        b_, h_ = c // 2, c % 2
        xTb = np.zeros((P, 2 * node_pad), np.float32)
        xTb[:, :half] = x[b_, :half].T
        xTb[:, node_pad : node_pad + half] = x[b_, half:].T
        m = dict(
            xT=xTb,
            xT_own=xTb[:, h_ * node_pad : (h_ + 1) * node_pad].copy(),
            W1a=W1a, W2a=W2a, W3a=W3a, b1T=b1, b2T=b2, b3T=b3,
            E4p=E4p, E1p=E1p, iota=iota,
        )
        m.update(idx_half[h_])
        in_maps.append(m)

    res = run_bass_kernel_spmd(nc, in_maps, core_ids=list(range(8)))

    out = np.zeros((Bc, Nn, N_CLS), np.float32)
    for c in range(8):
        b_, h_ = c // 2, c % 2
        o = res.results[c]["outT"]  # [N_CLS, infl]
        inv = pp["packs"][h_]["infl_of_node"]
        out[b_, h_ * half : (h_ + 1) * half] = o[:, inv].T
    return out



# revision 31
# speedup vs baseline: 1.9351x; 1.9351x over previous
"""Bass/Trainium2 kernel for 3-layer GAT over 8 NeuronCores.

Sharding: core 2b+h handles (batch b, dst-half h). Within a core:
  - Dense "table" matmuls produce per-node fp16 rows [edst|esrc|h|pad] for
    both halves (T_H0/T_H1), 128 fp16 cols = 256B gather rows.
  - Edges (dst-sorted, self-loops added) are packed into PAIRED bins of
    128 edge slots each: bin A holds a segment's half-0 sources, bin B its
    half-1 sources, <=15 segments per pair (slot 15 = dummy). Slot ids are
    "inflated" (16*pair+slot) so aggregation output columns are contiguous.
  - Per 32-pair batch: dma_gather by src from T_H0 (bin-A) and T_H1 (bin-B).
    The dst attention term e_dst is NOT gathered per edge: it is recomputed
    per SLOT with four tiny PE matmuls (a_dst-projection of the own-half
    activations in inflated slot order - a host-permuted x for layer 1, the
    previous layer's own output xn for layers 2/3), then expanded to edge
    positions with per-pair PE matmuls against precomputed transposed slot
    indicators. One extra identity matmul adds the gathered e_src, leaving
    z = e_src+e_dst in PSUM. s = exp(leakyrelu(z) - shift) via DVE max +
    ACT exp; the per-layer shift cancels in the softmax and keeps fp16 exp
    in range.
  - Aggregation folds into PE matmuls out[c,slot] = sum_e rhs[e,c]*onehot_e,
    rhs = [s*h | s] fp16, accumulating weighted sums + denominators in PSUM.
    Softmax max-subtraction is skipped (logits are O(10); the exp shift keeps
    fp16 in range) and matches the reference to rounding.
  - Normalization in the transposed layout (f32): reciprocal of denominators
    expands across head blocks with a tiny PE matmul; bias+relu fuse into one
    ACT op writing fp16. Output columns feed the next layer's table matmul
    directly; halves exchange via pairwise AllGather (fp16).
"""

import numpy as np

import concourse.bass as bass
import concourse.tile as tile
from concourse import bacc, mybir
from concourse.bass_utils import run_bass_kernel_spmd

F32 = mybir.dt.float32
F16 = mybir.dt.float16
I16 = mybir.dt.int16

NEG_SLOPE = 0.2
EDGE_DEPTH = 0
EPS = 1e-16
P = 128
NSEG_MAX = 15          # segments per pair (slot 15 reserved for dummies)
PAIRS_PER_BATCH = 32   # 4 supertiles of 8 pairs
TCOL = 128             # fp16 table row width (256B = min gather elem)
GCH = 1024             # gather chunk (SWDGE ring capacity limit per call)
MTILES = 8             # node tiles per phase_m round
EXP_SHIFT = [4.0, 0.0, 0.0]  # per-layer exp shift (cancels in softmax)

# Problem dims (hardcoded per the task contract)
N_NODES = 50000
B = 4
F_IN = 128
H, C = 4, 16
HC = H * C            # 64
N_CLS = 16


# ----------------------------------------------------------------------------
# Host preprocessing
# ----------------------------------------------------------------------------

def _pack_half(src, dst, n_lo, n_hi, half):
    """Pack one dst-half's edges into paired bins.

    Returns dict with per-pair arrays:
      srcA/srcB [np_, 128] global src node ids (bin A: src in half0)
      slotA/slotB [np_, 128] slot in 0..15 (15 = dummy)
      seg_node [np_, 16] dst node id of each slot (-1 unused)
    """
    half_n = n_hi - n_lo
    sel = (dst >= n_lo) & (dst < n_hi)
    s_, d_ = src[sel], dst[sel]
    order = np.argsort(d_, kind="stable")
    s_, d_ = s_[order], d_[order]
    uniq, seg_start = np.unique(d_, return_index=True)
    assert len(uniq) == half_n, "self-loops guarantee every node is a dst"
    seg_len = np.diff(np.append(seg_start, len(d_)))
    a_side = s_ < N_HALF_GLOBAL[0]  # bin A: src in global half 0
    pairs = []  # list of (list of seg ids)
    cur, curA, curB = [], 0, 0
    for i in range(half_n):
        a0, L = seg_start[i], seg_len[i]
        la = int(a_side[a0 : a0 + L].sum())
        lb = int(L - la)
        if len(cur) >= NSEG_MAX or curA + la > P or curB + lb > P:
            pairs.append(cur)
            cur, curA, curB = [], 0, 0
        cur.append(i)
        curA += la
        curB += lb
    if cur:
        pairs.append(cur)
    np_real = len(pairs)
    out = dict(np_real=np_real)
    npad = -(-np_real // PAIRS_PER_BATCH) * PAIRS_PER_BATCH
    srcA = np.zeros((npad, P), np.int64)
    srcB = np.full((npad, P), N_HALF_GLOBAL[0], np.int64)  # valid half-1 id
    slotA = np.full((npad, P), NSEG_MAX, np.int64)
    slotB = np.full((npad, P), NSEG_MAX, np.int64)
    seg_node = np.full((npad, 16), -1, np.int64)
    for k, segs in enumerate(pairs):
        ea = eb = 0
        for s_i, seg in enumerate(segs):
            a0, L = seg_start[seg], seg_len[seg]
            e_src = s_[a0 : a0 + L]
            e_a = e_src[a_side[a0 : a0 + L]]
            e_b = e_src[~a_side[a0 : a0 + L]]
            la, lb = len(e_a), len(e_b)
            srcA[k, ea : ea + la] = e_a
            slotA[k, ea : ea + la] = s_i
            srcB[k, eb : eb + lb] = e_b
            slotB[k, eb : eb + lb] = s_i
            seg_node[k, s_i] = uniq[seg]
            ea += la
            eb += lb
    out.update(srcA=srcA, srcB=srcB, slotA=slotA, slotB=slotB,
               seg_node=seg_node, npad=npad)
    return out


N_HALF_GLOBAL = [None]


def preprocess(edge_index, n_nodes):
    src = np.asarray(edge_index[0], np.int64)
    dst = np.asarray(edge_index[1], np.int64)
    loop = np.arange(n_nodes, dtype=np.int64)
    src = np.concatenate([src, loop])
    dst = np.concatenate([dst, loop])
    half = n_nodes // 2
    N_HALF_GLOBAL[0] = half
    packs = [_pack_half(src, dst, 0, half, 0),
             _pack_half(src, dst, half, n_nodes, 1)]
    npairs = max(p["npad"] for p in packs)
    npairs = -(-npairs // PAIRS_PER_BATCH) * PAIRS_PER_BATCH
    infl = 16 * npairs
    assert infl <= 32768, f"inflated id space {infl} exceeds int16 range"
    node_pad = -(-half // (MTILES * P)) * MTILES * P
    assert node_pad <= 32768
    for h, pk in enumerate(packs):
        k = npairs - pk["npad"]
        if k:
            for name, fill in [("srcA", 0), ("srcB", half),
                               ("slotA", NSEG_MAX), ("slotB", NSEG_MAX),
                               ("seg_node", -1)]:
                arr = pk[name]
                pad_shape = (k,) + arr.shape[1:]
                pk[name] = np.concatenate(
                    [arr, np.full(pad_shape, fill, arr.dtype)])
        # inflated id of each node (as a dst in its half)
        inv = np.full(half, -1, np.int64)
        sn = pk["seg_node"].reshape(-1)
        valid = sn >= 0
        inv[sn[valid] - h * half] = np.nonzero(valid)[0]
        assert (inv >= 0).all()
        pk["infl_of_node"] = inv  # [half] -> inflated id
    return dict(packs=packs, npairs=npairs, infl=infl, half=half,
                node_pad=node_pad, n_batches=npairs // PAIRS_PER_BATCH)


def _row_of_id(ids):
    """Table-row permutation: phase_m round q writes psum partition p,
    block j (source id q*1024 + j*128 + p) to row q*1024 + p*8 + j, so each
    SBUF partition writes 8 consecutive 256B rows = one 2KB descriptor."""
    ids = np.asarray(ids, np.int64)
    return (ids // 1024) * 1024 + (ids % 128) * MTILES + (ids % 1024) // 128


def _wrap_idx(flat):
    """dma_gather int16 index layout: idx i at [i%16, i//16], replicated to
    128 partitions."""
    n = len(flat)
    assert n % 16 == 0
    w = np.asarray(flat, np.int64).reshape(n // 16, 16).T
    assert w.max() < 32768 and w.min() >= -32768
    return np.tile(w.astype(np.int16), (8, 1))


def build_core_idx_arrays(pp, h):
    """Per-core (half h) host arrays, keyed by input-tensor name.

    srcA1/srcB1: [128, NB*256] i16 layer-1 half-local src ids (wrapped).
    srcA2/srcB2: same with inflated ids (layers 2/3).
    slAllA/slAllB: [128, NB*32] fp16 slot of each edge position per pair.
    itTA/itTB: [NB, 16, 4096] fp16 transposed slot indicators.
    """
    pk = pp["packs"][h]
    half = pp["half"]
    nb = pp["n_batches"]
    E_BLK = PAIRS_PER_BATCH * P  # 4096
    srcA = pk["srcA"].reshape(nb, E_BLK)
    srcB = pk["srcB"].reshape(nb, E_BLK)
    inflS = [pp["packs"][0]["infl_of_node"], pp["packs"][1]["infl_of_node"]]

    def loc(ids, src_half):
        return _row_of_id(ids - src_half * half)

    def infl_map(ids, src_half):
        return _row_of_id(inflS[src_half][ids - src_half * half])

    out = {}
    for tag, f in [("1", loc), ("2", infl_map)]:
        out[f"srcAB{tag}"] = np.concatenate(
            [np.concatenate([_wrap_idx(f(srcA[i], 0)),
                             _wrap_idx(f(srcB[i], 1))], axis=1)
             for i in range(nb)], axis=1)
    # slot of each edge position: [nb, pairs, P] -> [128, nb*pairs]
    for nm, sl in [("slAllA", pk["slotA"]), ("slAllB", pk["slotB"])]:
        v = sl.reshape(nb, PAIRS_PER_BATCH, P).transpose(2, 0, 1)
        out[nm] = v.reshape(P, nb * PAIRS_PER_BATCH).astype(np.float16)
    # transposed indicators: itT[k][s, g*128+p] = (slot[g, p] == s), A|B
    its = []
    for sl in (pk["slotA"], pk["slotB"]):
        s3 = sl.reshape(nb, PAIRS_PER_BATCH, P)  # [nb, g, p]
        it = (s3[:, None, :, :] == np.arange(16)[None, :, None, None])
        its.append(it.astype(np.float16).reshape(nb, 16, PAIRS_PER_BATCH * P))
    out["itTAB"] = np.concatenate(its, axis=2)
    return out


def augment_weights(W, a_s, a_d, tcol=TCOL):
    """[F, HC] weights -> fp16 [F, tcol] table weights [edst|esrc|h|0pad]."""
    Hh, Cc = a_s.shape
    W64 = np.asarray(W, np.float64)
    As = np.zeros((Hh * Cc, Hh))
    Ad = np.zeros((Hh * Cc, Hh))
    for hh in range(Hh):
        As[hh * Cc : (hh + 1) * Cc, hh] = np.asarray(a_s, np.float64)[hh]
        Ad[hh * Cc : (hh + 1) * Cc, hh] = np.asarray(a_d, np.float64)[hh]
    wa = np.concatenate([W64 @ Ad, W64 @ As, W64], axis=1)
    out = np.zeros((wa.shape[0], tcol), np.float16)
    out[:, : wa.shape[1]] = wa.astype(np.float16)
    return out


# ----------------------------------------------------------------------------
# Bass program
# ----------------------------------------------------------------------------

def build_program(node_pad, infl, n_batches, n_devices=8,
                  mock_collective=False, stop_after=None):
    """Build the SPMD bass program (identical on all cores)."""
    nc = bacc.Bacc("TRN2", target_bir_lowering=False, debug=False,
                   num_devices=n_devices)
    NB = n_batches
    E_BLK = PAIRS_PER_BATCH * P          # edges per side per batch (4096)
    IDXC = E_BLK // 16                   # idx cols for 4096 idxs (256)
    SLC = PAIRS_PER_BATCH                # 32 slot cols per batch

    ins = {}

    def inp(name, shape, dtype=F32):
        ins[name] = nc.dram_tensor(name, list(shape), dtype,
                                   kind="ExternalInput")
        return ins[name]

    xT = inp("xT", [P, 2 * node_pad], F16)      # both halves, transposed
    inp("xSl", [P, infl], F16)                  # own-half x in slot order
    inp("W1a", [F_IN, TCOL], F16)
    inp("W2a", [HC, TCOL], F16)
    inp("W3a", [HC, TCOL], F16)
    inp("b1T", [HC, 1])
    inp("b2T", [HC, 1])
    inp("b3T", [N_CLS, 1])
    inp("e4s", [H, HC])                         # head indicator [4, 64]
    inp("e1s", [1, N_CLS])                      # ones row
    inp("iota", [P, 16], F16)
    inp("ident", [P, P], F16)
    inp("shm", [P, 1])                          # -EXP_SHIFT[0] column
    for t in ("1", "2"):
        inp(f"srcAB{t}", [P, NB * 2 * IDXC], I16)
    inp("slAllA", [P, NB * SLC], F16)
    inp("slAllB", [P, NB * SLC], F16)
    inp("itTAB", [NB, 16, 2 * E_BLK], F16)
    outT = nc.dram_tensor("outT", [N_CLS, infl], F32, kind="ExternalOutput")

    GROUPS = [[2 * b_ + 0, 2 * b_ + 1] for b_ in range(n_devices // 2)]

    with tile.TileContext(nc) as tc:
        with (
            tc.tile_pool(name="dram", bufs=1, space="DRAM") as dp,
            tc.tile_pool(name="const", bufs=1) as cp,
            tc.tile_pool(name="idx", bufs=1) as ip,
            tc.tile_pool(name="mm", bufs=3) as mp,
            tc.tile_pool(name="edge", bufs=2) as ep,
            tc.tile_pool(name="norm", bufs=2) as np_,
            tc.tile_pool(name="psm", bufs=2, space="PSUM") as ps_m,
            tc.tile_pool(name="pse", bufs=2, space="PSUM") as ps_e,
            tc.tile_pool(name="psd", bufs=2, space="PSUM") as ps_d,
        ):
            # DRAM intermediates
            T1 = [dp.tile([node_pad, TCOL], F16, tag=f"T1{h}", name=f"T1{h}")
                  for h in range(2)]
            T2 = [dp.tile([infl, TCOL], F16, tag=f"T2{h}", name=f"T2{h}")
                  for h in range(2)]
            T3 = [dp.tile([infl, TCOL], F16, tag=f"T3{h}", name=f"T3{h}")
                  for h in range(2)]
            xn = [dp.tile([HC, infl], F16, tag=f"xn{l}", name=f"xn{l}")
                  for l in range(2)]
            xnf = [dp.tile([2 * HC, infl], F16, tag=f"xnf{l}", name=f"xnf{l}")
                   for l in range(2)]

            # constants
            w1_t = cp.tile([F_IN, TCOL], F16)
            w2_t = cp.tile([HC, TCOL], F16)
            w3_t = cp.tile([HC, TCOL], F16)
            b1_t = cp.tile([HC, 1], F32)
            b2_t = cp.tile([HC, 1], F32)
            b3_t = cp.tile([N_CLS, 1], F32)
            e4_t = cp.tile([H, HC], F32)
            e1_t = cp.tile([1, N_CLS], F32)
            io_t = cp.tile([P, 16], F16)
            id_t = cp.tile([P, P], F16)
            sh_t = cp.tile([P, 1], F32)
            for t_, d_ in [(w1_t, ins["W1a"]), (w2_t, ins["W2a"]),
                           (w3_t, ins["W3a"]), (b1_t, ins["b1T"]),
                           (b2_t, ins["b2T"]), (b3_t, ins["b3T"]),
                           (e4_t, ins["e4s"]), (e1_t, ins["e1s"]),
                           (io_t, ins["iota"]), (id_t, ins["ident"]),
                           (sh_t, ins["shm"])]:
                nc.sync.dma_start(out=t_[:], in_=d_[:, :])
            # static per-edge slot arrays (all layers)
            slA_t = ip.tile([P, NB * SLC], F16, tag="slA")
            slB_t = ip.tile([P, NB * SLC], F16, tag="slB")
            nc.sync.dma_start(out=slA_t[:], in_=ins["slAllA"][:, :])
            nc.sync.dma_start(out=slB_t[:], in_=ins["slAllB"][:, :])

            def phase_m(dst_tables, src_full, w_t, kdim):
                """Dense table matmuls, MTILES node-tiles per round.
                src_full: AP-maker f(h) -> [kdim, *] fp16."""
                n_t = dst_tables[0].shape[0] // P
                assert n_t % MTILES == 0
                MB = MTILES * P

                def rnd(src_ap, q, table):
                    xc = mp.tile([kdim, MB], F16, tag="xc")
                    nc.sync.dma_start(out=xc[:],
                                      in_=src_ap[:, q * MB : (q + 1) * MB])
                    psm = ps_m.tile([P, MTILES * TCOL], F32, space="PSUM",
                                    tag="psm")
                    for j in range(MTILES):
                        nc.tensor.matmul(
                            out=psm[:, j * TCOL : (j + 1) * TCOL],
                            lhsT=xc[:, j * P : (j + 1) * P], rhs=w_t[:],
                            start=True, stop=True)
                    sb = mp.tile([P, MTILES * TCOL], F16, tag="msb")
                    nc.vector.tensor_copy(out=sb[:], in_=psm[:])
                    # row q*1024 + p*8 + j: each partition writes 8
                    # consecutive rows = one contiguous 2KB descriptor
                    nc.sync.dma_start(
                        out=table[:][q * MB : (q + 1) * MB, :].rearrange(
                            "(r j) c -> r j c", j=MTILES),
                        in_=sb[:].rearrange("p (j c) -> p j c", c=TCOL))

                for h in range(2):
                    for q in range(n_t // MTILES):
                        rnd(src_full(h), q, dst_tables[h])

            def edge_phase(layer, tabs, idx_tag, nheads, ncols,
                           bias_t, exp_t, relu, ed_src, ed_w, ed_kdim,
                           out_dst):
                """One layer's edge aggregation.

                nheads: H or 1; ncols: C or N_CLS (per-head out width)
                ed_src: AP maker f(k) -> [ed_kdim, 512] slot-ordered
                    activations in DRAM; ed_w: [ed_kdim, nheads] a_dst
                    projection columns (fp16 SBUF)
                out_dst: [nheads*ncols, infl] DRAM AP for layer outputs
                """
                eoff = nheads          # esrc col offset in table row
                out_cols = nheads * ncols
                # denominator rows must sit at a partition base that is a
                # multiple of 32 (BIR verifier): pad L3's 16 s*h rows to 32
                dn0 = -(-out_cols // 32) * 32
                RW = dn0 + nheads      # [s*h | pad | s] lhs cols
                shift = EXP_SHIFT[layer - 1]
                sl_t = {"a": slA_t, "b": slB_t}
                soff = {"a": 0, "b": IDXC}
                toff = {"a": 0, "b": E_BLK}
                for k in range(NB):
                    # shared PSUM bank: [zA | zB | edp] column regions
                    zp3 = ps_d.tile([P, 3 * P], F32, space="PSUM", tag="zp3")
                    zpr = {"a": zp3[:, 0 : PAIRS_PER_BATCH * nheads],
                           "b": zp3[:, P : P + PAIRS_PER_BATCH * nheads]}
                    edp = zp3[0:16, 2 * P : 2 * P + PAIRS_PER_BATCH * nheads]
                    # --- per-slot e_dst via PE, transposed: [16 slots, g*nh]
                    xsl = ep.tile([ed_kdim, 512], F16, tag=f"xsl{ed_kdim}")
                    nc.sync.dma_start(out=xsl[:], in_=ed_src(k))
                    for g in range(PAIRS_PER_BATCH):
                        nc.tensor.matmul(
                            out=edp[:, g * nheads : (g + 1) * nheads],
                            lhsT=xsl[:, g * 16 : (g + 1) * 16], rhs=ed_w,
                            start=True, stop=True)
                    eds = ep.tile([16, PAIRS_PER_BATCH * nheads], F16,
                                  tag="eds")
                    nc.scalar.activation(
                        out=eds[:], in_=edp[:],
                        func=mybir.ActivationFunctionType.Identity)
                    # --- src idx + gathers
                    sx = ep.tile([P, 2 * IDXC], I16, tag="sx")
                    nc.sync.dma_start(
                        out=sx[:],
                        in_=ins[f"srcAB{idx_tag}"][
                            :, k * 2 * IDXC : (k + 1) * 2 * IDXC])
                    itT = ep.tile([16, 2 * E_BLK], F16, tag="itT", bufs=1)
                    nc.sync.dma_start(out=itT[:], in_=ins["itTAB"][k])
                    g_t = {}
                    for side, tab in (("a", tabs[0]), ("b", tabs[1])):
                        gS = ep.tile([P, PAIRS_PER_BATCH * TCOL], F16,
                                     tag=f"g{side}")
                        for g0 in range(0, E_BLK, GCH):
                            blk = slice(g0 // P, (g0 + GCH) // P)
                            col = slice(soff[side] + g0 // 16,
                                        soff[side] + (g0 + GCH) // 16)
                            nc.gpsimd.dma_gather(
                                out_ap=gS[:].rearrange(
                                    "p (g c) -> p g c", c=TCOL)[:, blk, :],
                                in_ap=tab[:][:, :], idxs_ap=sx[:, col],
                                num_idxs=GCH, num_idxs_reg=GCH,
                                elem_size=TCOL, elem_step=TCOL, queue_num=0)
                        g_t[side] = gS
                    if EDGE_DEPTH == 1:
                        continue
                    rhs = {}
                    ind = {}
                    for side in ("a", "b"):
                        gv = g_t[side][:].rearrange("p (g c) -> p g c",
                                                    c=TCOL)
                        # z = e_src + e_dst in PSUM via PE
                        zps = zpr[side]
                        # start=True zeroes the whole 2KB psum bank: the
                        # full-region e_src identity matmul must come FIRST;
                        # per-pair e_dst expansions then accumulate into it.
                        nc.tensor.matmul(
                            out=zps[:],
                            lhsT=id_t[:],
                            rhs=gv[:, :, eoff : eoff + nheads],
                            start=True, stop=False, skip_group_check=True)
                        for g in range(PAIRS_PER_BATCH):
                            nc.tensor.matmul(
                                out=zps[:, g * nheads : (g + 1) * nheads],
                                lhsT=itT[:, toff[side] + g * P
                                         : toff[side] + (g + 1) * P],
                                rhs=eds[:, g * nheads : (g + 1) * nheads],
                                start=False, stop=(g == PAIRS_PER_BATCH - 1),
                                skip_group_check=True)
                        # s = exp(max(z, 0.2z) - shift), fp16
                        zs = ep.tile([P, PAIRS_PER_BATCH * nheads], F32,
                                     tag=f"zs{side}")
                        nc.vector.tensor_scalar_mul(out=zs[:], in0=zps[:],
                                                    scalar1=NEG_SLOPE)
                        z_t = ep.tile([P, PAIRS_PER_BATCH * nheads], F16,
                                      tag=f"z{side}")
                        nc.vector.tensor_tensor(out=z_t[:], in0=zs[:],
                                                in1=zps[:],
                                                op=mybir.AluOpType.max)
                        s_t = ep.tile([P, PAIRS_PER_BATCH * nheads], F16,
                                      tag=f"s{side}")
                        if shift:
                            nc.scalar.activation(
                                out=s_t[:], in_=z_t[:],
                                func=mybir.ActivationFunctionType.Exp,
                                bias=sh_t[:, :1], scale=1.0)
                        else:
                            nc.scalar.activation(
                                out=s_t[:], in_=z_t[:],
                                func=mybir.ActivationFunctionType.Exp)
                        # rhs = [s*h | s] fp16
                        r = ep.tile([P, PAIRS_PER_BATCH * RW], F16,
                                    tag=f"r{side}")
                        rv = r[:].rearrange("p (g c) -> p g c", c=RW)
                        sv = s_t[:].rearrange("p (g h) -> p g h", h=nheads)
                        nc.vector.tensor_tensor(
                            out=rv[:, :, : nheads * ncols].rearrange(
                                "p g (h c) -> p g h c", c=ncols),
                            in0=gv[:, :, 2 * nheads : 2 * nheads
                                   + nheads * ncols].rearrange(
                                "p g (h c) -> p g h c", c=ncols),
                            in1=sv.unsqueeze(3).broadcast_to(
                                [P, PAIRS_PER_BATCH, nheads, ncols]),
                            op=mybir.AluOpType.mult)
                        if dn0 > out_cols:
                            nc.vector.memset(rv[:, :, out_cols:dn0], 0.0)
                        nc.vector.tensor_copy(out=rv[:, :, dn0:RW], in_=sv)
                        # slot indicator for aggregation
                        it = ep.tile([P, PAIRS_PER_BATCH * 16], F16,
                                     tag=f"i{side}")
                        nc.vector.tensor_tensor(
                            out=it[:].rearrange("p (g i) -> p g i", i=16),
                            in0=sl_t[side][:, k * SLC : (k + 1) * SLC]
                                .unsqueeze(2).broadcast_to(
                                    [P, PAIRS_PER_BATCH, 16]),
                            in1=io_t[:].unsqueeze(1).broadcast_to(
                                [P, PAIRS_PER_BATCH, 16]),
                            op=mybir.AluOpType.is_equal)
                        rhs[side] = r
                        ind[side] = it
                    if EDGE_DEPTH == 2:
                        continue
                    o_st = np_.tile([out_cols, 4 * P], F32, tag="ost")
                    for st in range(PAIRS_PER_BATCH // 8):
                        nrm = ps_e.tile([RW, 2 * P], F32, space="PSUM",
                                        tag="nrm")
                        acc = nrm[:, 0:P]
                        for j in range(8):
                            g = st * 8 + j
                            nc.tensor.matmul(
                                out=acc[:, 16 * j : 16 * (j + 1)],
                                lhsT=rhs["a"][:, RW * g : RW * (g + 1)],
                                rhs=ind["a"][:, 16 * g : 16 * (g + 1)],
                                start=True, stop=False)
                            nc.tensor.matmul(
                                out=acc[:, 16 * j : 16 * (j + 1)],
                                lhsT=rhs["b"][:, RW * g : RW * (g + 1)],
                                rhs=ind["b"][:, 16 * g : 16 * (g + 1)],
                                start=False, stop=True)
                        if EDGE_DEPTH == 3:
                            continue
                        seg = np_.tile([RW, P], F32, tag="seg")
                        nc.scalar.activation(
                            out=seg[:], in_=acc[:],
                            func=mybir.ActivationFunctionType.Identity)
                        rec = np_.tile([nheads, P], F32, tag="rec")
                        nc.vector.tensor_scalar_add(
                            out=rec[:], in0=seg[dn0 : dn0 + nheads, :],
                            scalar1=EPS)
                        nc.vector.reciprocal(out=rec[:], in_=rec[:])
                        rxp = nrm[0:out_cols, P : 2 * P]
                        nc.tensor.matmul(out=rxp, lhsT=exp_t[:],
                                         rhs=rec[:], start=True, stop=True)
                        nc.vector.tensor_tensor(
                            out=o_st[:, st * P : (st + 1) * P],
                            in0=seg[:out_cols, :], in1=rxp,
                            op=mybir.AluOpType.mult)
                    if EDGE_DEPTH == 3:
                        continue
                    o_t = np_.tile(
                        [out_cols, 4 * P], F32 if layer == 3 else F16,
                        tag="o")
                    nc.scalar.activation(
                        out=o_t[:], in_=o_st[:],
                        func=(mybir.ActivationFunctionType.Relu if relu
                              else mybir.ActivationFunctionType.Identity),
                        bias=bias_t[:, :1], scale=1.0)
                    col0 = k * 4 * P
                    nc.sync.dma_start(
                        out=out_dst[:, col0 : col0 + 4 * P], in_=o_t[:])

            # ---------------- layer 1 ----------------
            phase_m(T1, lambda h: xT[:, h * node_pad : (h + 1) * node_pad],
                    w1_t, F_IN)
            stop = [False]

            def _chk(tag):
                if stop[0] or stop_after == tag:
                    stop[0] = True
                return stop[0]

            if not _chk("m1"):
                edge_phase(1, T1, "1", H, C, b1_t, e4_t, True,
                           lambda k: ins["xSl"][:, k * 512 : (k + 1) * 512],
                           w1_t[:, 0:H], F_IN, xn[0][:])
            if not _chk("e1"):
                if mock_collective:
                    nc.sync.dma_start(out=xnf[0][:][:HC, :], in_=xn[0][:][:, :])
                    nc.sync.dma_start(out=xnf[0][:][HC:, :], in_=xn[0][:][:, :])
                else:
                    nc.gpsimd.collective_compute(
                        "AllGather", mybir.AluOpType.bypass,
                        replica_groups=GROUPS,
                        ins=[xn[0][:][:, :]], outs=[xnf[0][:][:, :]])
            # ---------------- layer 2 ----------------
            xnf0 = xnf[0][:]
            if not _chk("x1"):
                phase_m(T2, lambda h: xnf0[h * HC : (h + 1) * HC, :],
                        w2_t, HC)
            if not _chk("m2"):
                edge_phase(2, T2, "2", H, C, b2_t, e4_t, True,
                           lambda k: xn[0][:][:, k * 512 : (k + 1) * 512],
                           w2_t[:, 0:H], HC, xn[1][:])
            if not _chk("e2"):
                if mock_collective:
                    nc.sync.dma_start(out=xnf[1][:][:HC, :], in_=xn[1][:][:, :])
                    nc.sync.dma_start(out=xnf[1][:][HC:, :], in_=xn[1][:][:, :])
                else:
                    nc.gpsimd.collective_compute(
                        "AllGather", mybir.AluOpType.bypass,
                        replica_groups=GROUPS,
                        ins=[xn[1][:][:, :]], outs=[xnf[1][:][:, :]])
            # ---------------- layer 3 ----------------
            xnf1 = xnf[1][:]
            if not _chk("x2"):
                phase_m(T3, lambda h: xnf1[h * HC : (h + 1) * HC, :],
                        w3_t, HC)
            if not _chk("m3"):
                edge_phase(3, T3, "2", 1, N_CLS, b3_t, e1_t, False,
                           lambda k: xn[1][:][:, k * 512 : (k + 1) * 512],
                           w3_t[:, 0:1], HC, outT[:, :])

    nc.compile()
    return nc


# ----------------------------------------------------------------------------
# Entry point
# ----------------------------------------------------------------------------

BUILD_KWARGS = {}

def kernel(**inputs):
    x = np.asarray(inputs["x"], np.float32)
    edge_index = np.asarray(inputs["edge_index"])
    Bc, Nn, Fi = x.shape
    pp = preprocess(edge_index, Nn)
    half, node_pad, infl = pp["half"], pp["node_pad"], pp["infl"]

    W1a = augment_weights(inputs["W1"], inputs["a1s"], inputs["a1d"])
    W2a = augment_weights(inputs["W2"], inputs["a2s"], inputs["a2d"])
    W3a = augment_weights(inputs["W3"], inputs["a3s"], inputs["a3d"])
    b1 = np.asarray(inputs["b1"], np.float32).reshape(HC, 1)
    b2 = np.asarray(inputs["b2"], np.float32).reshape(HC, 1)
    b3 = np.asarray(inputs["b3"], np.float32).reshape(N_CLS, 1)
    e4s = np.zeros((H, HC), np.float32)
    for hh in range(H):
        e4s[hh, hh * C : (hh + 1) * C] = 1.0
    e1s = np.ones((1, N_CLS), np.float32)
    iota = np.broadcast_to(np.arange(16, dtype=np.float16), (P, 16)).copy()
    ident = np.eye(P, dtype=np.float16)
    shm = np.full((P, 1), -EXP_SHIFT[0], np.float32)

    nc = build_program(node_pad, infl, pp["n_batches"], n_devices=8,
                       **BUILD_KWARGS)

    # per-core inputs
    idx_half = [build_core_idx_arrays(pp, h) for h in range(2)]
    in_maps = []
    for c in range(8):
        b_, h_ = c // 2, c % 2
        xTb = np.zeros((P, 2 * node_pad), np.float16)
        xTb[:, :half] = x[b_, :half].T
        xTb[:, node_pad : node_pad + half] = x[b_, half:].T
        # own-half x columns in inflated slot order (dummy slots -> 0)
        sn = pp["packs"][h_]["seg_node"].reshape(-1)  # [infl] global ids
        xSl = np.zeros((P, infl), np.float16)
        valid = sn >= 0
        xSl[:, valid] = x[b_][sn[valid]].T
        m = dict(
            xT=xTb, xSl=xSl,
            W1a=W1a, W2a=W2a, W3a=W3a, b1T=b1, b2T=b2, b3T=b3,
            e4s=e4s, e1s=e1s, iota=iota, ident=ident, shm=shm,
        )
        m.update(idx_half[h_])
        in_maps.append(m)

    res = run_bass_kernel_spmd(nc, in_maps, core_ids=list(range(8)))

    out = np.zeros((Bc, Nn, N_CLS), np.float32)
    for c in range(8):
        b_, h_ = c // 2, c % 2
        o = res.results[c]["outT"]  # [N_CLS, infl]
        inv = pp["packs"][h_]["infl_of_node"]
        out[b_, h_ * half : (h_ + 1) * half] = o[:, inv].T
    return out
